# revision 23
# baseline (speedup 1.0000x reference)
"""CLIP encoder layer on 8 trn2 NeuronCores, pure data parallel over batch.

Layout strategy (per core, batch shard of 64 sequences = 4928 tokens):
  - x arrives token-major [T, 768] fp32.
  - LayerNorm runs token-major (tokens on partitions, bn_stats/bn_aggr),
    LN scale/bias folded into the downstream projection weights host-side.
    rstd computed as exp(-0.5*ln(var+eps)) on ACT so the LN path shares the
    natural_log_exp table set with attention's exp (no Sqrt table swaps,
    no DVE reciprocal).
  - Normalized activations are PE-transposed (bf16) to feature-major
    [768, N] for the projections (weights stationary, activations moving).
  - q/k PSUM drains (bias add + bf16 cast) run on ACT (Identity+bias),
    freeing DVE.
  - Attention per sequence (S=77), software-pipelined across the 4
    sequences of a superblock so the PE always has independent matmuls
    queued while softmax chains run on ACT/DVE:
      iter s: v_s, scores_s (6 matmuls/half into one PSUM bank) + one
      batched exp per half; tr_{s-1} (6 transposes into one PSUM bank,
      one batched copy); ctx_{s-2} (head pairs share a [128,77] PSUM tile
      via partition-offset writes, one cast per pair).
    Softmax: no max subtraction (scores bounded ~|2.5|); causal mask
    applied multiplicatively after exp.
  - O-projection runs with swapped operands (activations stationary) so
    its output comes out token-major, letting the residual add and LN2
    stay token-major with no full-tensor transposes.
  - FC2 runs feature-major (weights stationary, ff1 moving): 709k PE
    cycles instead of 884k; bias added during the ACT Identity drain
    (per-partition), then PE-transposed back to token-major for the
    residual add.
  - All matmuls in bf16 (fp32 PSUM accumulation); fp32 elsewhere.
    QuickGELU via ACT Silu: x*sigmoid(1.702x) = silu(1.702x)/1.702 with the
    1/1.702 folded into fc2 weights and the 1.702 into the ACT input scale.
"""

import os
import numpy as np
import ml_dtypes

D = 768
H = 12
HD = 64
S = 77
FF = 3072
EPS = 1e-5
N_CORES = 8
B_FULL = 512
BPC = B_FULL // N_CORES          # 64 sequences per core
T_CORE = BPC * S                 # 4928 tokens per core
G_SEQ = 4                        # sequences per superblock
SB = G_SEQ * S                   # 308 tokens per superblock


def build_program(T=T_CORE, G=G_SEQ, use_silu=True, stages="ABCDEF"):
    import concourse.bass as bass
    import concourse.bacc as bacc
    import concourse.mybir as mybir
    import concourse.tile as tile
    from concourse.masks import make_identity
    from contextlib import ExitStack

    f32 = mybir.dt.float32
    bf16 = mybir.dt.bfloat16
    f8 = mybir.dt.float8e4
    AX = mybir.AxisListType
    OP = mybir.AluOpType
    AF = mybir.ActivationFunctionType

    SBLK = G * S
    NSB = T // SBLK
    assert NSB * SBLK == T
    # token chunks within a superblock
    chunks = []
    off = 0
    while off < SBLK:
        w = min(128, SBLK - off)
        chunks.append((off, w))
        off += w

    nc = bacc.Bacc("TRN2", target_bir_lowering=False)

    x_d = nc.declare_dram_parameter("x", [T, D], f32, isOutput=False)
    wq_d = nc.declare_dram_parameter("wqT", [128, D // 256, 2, D], f8,
                                     isOutput=False)
    wk_d = nc.declare_dram_parameter("wkT", [128, D // 256, 2, D], f8,
                                     isOutput=False)
    wv_d = nc.declare_dram_parameter("wvT", [128, D // 256, 2, D], f8,
                                     isOutput=False)
    wo_d = nc.declare_dram_parameter("woT", [D, D], bf16, isOutput=False)
    # fc1/fc2 weights arrive fp8 (e4m3, scaled x64 / x128) pre-packed for
    # DoubleRow: [p, blk, i, out] with contraction index k = blk*256+i*128+p
    wf1_d = nc.declare_dram_parameter("fc1T", [128, D // 256, 2, FF], f8,
                                      isOutput=False)
    wf2_d = nc.declare_dram_parameter("fc2T", [128, FF // 256, 2, D], f8,
                                      isOutput=False)
    qb_d = nc.declare_dram_parameter("qb", [D], f32, isOutput=False)
    kb_d = nc.declare_dram_parameter("kb", [D], f32, isOutput=False)
    vb_d = nc.declare_dram_parameter("vb", [D], f32, isOutput=False)
    ob_d = nc.declare_dram_parameter("ob", [D], f32, isOutput=False)
    f1b_d = nc.declare_dram_parameter("fc1b", [FF], f32, isOutput=False)
    f2b_d = nc.declare_dram_parameter("fc2b", [D], f32, isOutput=False)
    mask_d = nc.declare_dram_parameter("mask", [S, S], bf16, isOutput=False)
    out_d = nc.declare_dram_parameter("out", [T, D], f32, isOutput=True)

    with tile.TileContext(nc) as tc, ExitStack() as ctx:
        singles = ctx.enter_context(tc.tile_pool(name="singles", bufs=1))
        xpool = ctx.enter_context(tc.tile_pool(name="xpool", bufs=3))
        x2pool = ctx.enter_context(tc.tile_pool(name="x2pool", bufs=3))
        actpool = ctx.enter_context(tc.tile_pool(name="actpool", bufs=1))
        outpool = ctx.enter_context(tc.tile_pool(name="outpool", bufs=2))
        attnpool = ctx.enter_context(tc.tile_pool(name="attnpool", bufs=2))
        statpool = ctx.enter_context(tc.tile_pool(name="statpool", bufs=2))
        pspool = ctx.enter_context(tc.tile_pool(name="pspool", bufs=1, space="PSUM"))

        NCH = D // 128    # 6
        NFF = FF // 128   # 24

        # ---- tiny constants first (cheap DMAs, unblock LN immediately) ----
        qb_sb = singles.tile([128, D // 128], f32)
        kb_sb = singles.tile([128, D // 128], f32)
        vb_sb = singles.tile([128, D // 128], f32)
        f1b_sb = singles.tile([128, FF // 128], f32)
        f2b_sb = singles.tile([128, D // 128], f32)
        for sb_t, dr in ((qb_sb, qb_d), (kb_sb, kb_d), (vb_sb, vb_d),
                         (f1b_sb, f1b_d), (f2b_sb, f2b_d)):
            nc.sync.dma_start(out=sb_t, in_=dr[:].rearrange("(c p) -> p c", p=128))

        # free-axis bias broadcast to all 128 partitions (DMAs issued
        # after stage A(0)'s x loads; only needed from stage C on)
        ob_bc = singles.tile([128, D], f32)
        mask_sb = singles.tile([S, S], bf16)

        def load_bcast():
            srcap = bass.AP(tensor=ob_d[:].tensor, offset=ob_d[:].offset,
                            ap=[[0, 128]] + list(ob_d[:].ap))
            nc.sync.dma_start(out=ob_bc, in_=srcap)
            nc.sync.dma_start(out=mask_sb, in_=mask_d[:])

        ident = singles.tile([128, 128], bf16)
        make_identity(nc, ident)



        # ---- weights (declared up front, DMAs issued after stage A(0)) ----
        wq_sb = singles.tile([128, D // 256, 2, D], f8)
        wk_sb = singles.tile([128, D // 256, 2, D], f8)
        wv_sb = singles.tile([128, D // 256, 2, D], f8)
        wo_sb = singles.tile([128, D // 128, D], bf16)
        wf1_sb = singles.tile([128, D // 256, 2, FF], f8)
        wf2_sb = singles.tile([128, FF // 256, 2, D], f8)

        def load_weights():
            for sb_t, dr in ((wq_sb, wq_d), (wk_sb, wk_d), (wv_sb, wv_d)):
                nc.sync.dma_start(out=sb_t, in_=dr[:])
            nc.sync.dma_start(
                out=wo_sb, in_=wo_d[:].rearrange("(c p) o -> p c o", p=128))
            nc.sync.dma_start(out=wf1_sb, in_=wf1_d[:])
            nc.sync.dma_start(out=wf2_sb, in_=wf2_d[:])

        i32 = mybir.dt.int32
        MAGIC1 = 0x5F3759DF + 1

        def ln_start(src_tile, w, tag):
            """bn stats for a token-major [w, 768] fp32 tile (cheap, emit early)."""
            stats = statpool.tile([128, 3, 6], f32, tag=f"stats{tag}", name=f"stats{tag}")
            mv = statpool.tile([128, 2], f32, tag=f"mv{tag}", name=f"mv{tag}",
                               bufs=3)
            xg = src_tile[:w].rearrange("p (s f) -> p s f", f=256)
            for i in range(3):
                nc.vector.bn_stats(out=stats[:w, i, :], in_=xg[:, i, :])
            nc.vector.bn_aggr(out=mv[:w], in_=stats[:w])
            return mv

        def ln_finish(mv, src_tile, w, tag, bufs=2, scale16=False):
            """rstd via DVE-only Newton rsqrt (no ACT table), then normalize.
            scale16 folds a x16 into rstd so fp8 outputs use more range."""
            mean = mv[:w, 0:1]
            nt = statpool.tile([128, 4], f32, tag=f"nt{tag}", name=f"nt{tag}",
                               bufs=3)
            v = nt[:w, 0:1]     # var + eps
            y = nt[:w, 1:2]     # rsqrt estimate
            a = nt[:w, 2:3]
            b = nt[:w, 3:4]
            nc.vector.tensor_scalar(out=v, in0=mv[:w, 1:2], scalar1=float(EPS),
                                    scalar2=None, op0=OP.add)
            vi = v.bitcast(i32)
            yi = y.bitcast(i32)
            nc.vector.tensor_scalar(out=yi, in0=vi, scalar1=1, scalar2=None,
                                    op0=OP.arith_shift_right)
            nc.vector.tensor_scalar(out=yi, in0=yi, scalar1=-1, scalar2=None,
                                    op0=OP.bitwise_xor)
            nc.vector.tensor_scalar(out=yi, in0=yi, scalar1=MAGIC1, scalar2=None,
                                    op0=OP.add)
            for it in range(2):
                last = it == 1
                s2, s3 = (-8.0, 24.0) if (scale16 and last) else (-0.5, 1.5)
                nc.vector.tensor_tensor(out=a, in0=y, in1=y, op=OP.mult)
                nc.vector.tensor_tensor(out=b, in0=v, in1=a, op=OP.mult)
                nc.vector.tensor_scalar(out=b, in0=b, scalar1=s2, scalar2=s3,
                                        op0=OP.mult, op1=OP.add)
                nc.vector.tensor_tensor(out=y, in0=y, in1=b, op=OP.mult)
            htok = statpool.tile([128, D], bf16, tag=f"htok{tag}", name=f"htok{tag}",
                                 bufs=bufs)
            nc.vector.tensor_scalar(out=htok[:w], in0=src_tile[:w],
                                    scalar1=mean, scalar2=y,
                                    op0=OP.subtract, op1=OP.mult)
            return htok

        def ln_transpose(htok, coff, w, hT, tag):
            for c in range(NCH):
                ps = bf_ps(f"trp{tag}")
                nc.tensor.transpose(ps[:, :w], htok[:w, c * 128:(c + 1) * 128],
                                    ident[:w, :w])
                nc.any.tensor_copy(out=hT[:, c, coff:coff + w], in_=ps[:, :w])

        def stage_A_start(isb):
            """load x chunks + bn stats (no ACT, cheap to emit early)."""
            t0 = isb * SBLK
            x_tiles, mvs = [], []
            for (coff, w) in chunks:
                x_tok = xpool.tile([128, D], f32, tag="xtok", name="xtok")
                nc.sync.dma_start(out=x_tok[:w], in_=x_d[t0 + coff: t0 + coff + w, :])
                x_tiles.append(x_tok)
                mvs.append(ln_start(x_tok, w, "A"))
            return x_tiles, mvs

        def stage_A_finish(x_tiles, mvs):
            """LN1 -> hT feature-major fp8 (x16)."""
            hT = actpool.tile([128, NCH, SBLK], f8, tag="hT", name="hT", bufs=2)
            htoks = []
            for ci, (coff, w) in enumerate(chunks):
                htoks.append(ln_finish(mvs[ci], x_tiles[ci], w, "A", bufs=3,
                                       scale16=True))
            for ci, (coff, w) in enumerate(chunks):
                ln_transpose(htoks[ci], coff, w, hT, "A")
            return hT

        # All PSUM tiles are sized to exactly one 2KB bank so every tile is
        # bank-aligned (matmul outputs must not cross a bank boundary).
        def big_ps(name):
            return pspool.tile([128, 512], f32, tag="big", name=name, bufs=3)

        def bf_ps(name):
            # shared bf16 PSUM bank for transpose drains (attn, LN, fc2)
            return pspool.tile([128, 1024], bf16, tag="bfps", name=name, bufs=2)

        def stage_D_chunk(ci, ctxT, x_tiles, x2_tiles):
            coff, w = chunks[ci]
            x2 = x2pool.tile([128, D], f32, tag="x2tok", name="x2tok")
            for half in range(2):
                ps = big_ps("pso")
                for d in range(NCH):
                    nc.tensor.matmul(ps[:w, :384], lhsT=ctxT[:, d, coff:coff + w],
                                     rhs=wo_sb[:, d, half * 384:(half + 1) * 384],
                                     start=(d == 0), stop=(d == NCH - 1))
                sl = slice(half * 384, (half + 1) * 384)
                nc.vector.tensor_tensor(out=x2[:w, sl], in0=ps[:w, :384],
                                        in1=ob_bc[:w, sl], op=OP.add)
                nc.vector.tensor_tensor(out=x2[:w, sl], in0=x2[:w, sl],
                                        in1=x_tiles[ci][:w, sl], op=OP.add)
            x2_tiles.append(x2)

        a_parts = stage_A_start(0)
        load_bcast()
        load_weights()
        cur = (stage_A_finish(*a_parts), a_parts[0])
        for isb in range(NSB):
            t0 = isb * SBLK
            hT, x_tiles = cur

            # ---- stage B: q/k projections (feature-major, bf16) ----
            # per-head layout [64, H, SBLK]: every scores lhsT starts at
            # partition 0 (a matmul with lhsT at partition base 64 AND a
            # free-offset PSUM output hangs the device)
            qT = actpool.tile([64, H, SBLK], bf16, tag="qT", name="qT")
            kT = actpool.tile([64, H, SBLK], bf16, tag="kT", name="kT")
            DR = mybir.MatmulPerfMode.DoubleRow
            # psum carries x1024 (x16 hT, x64 weights); q's 0.125 is applied
            # in the drain scale
            for dst, w_sb, b_sb, dsc in ((qT, wq_sb, qb_sb, 0.125 / 1024.0),
                                         (kT, wk_sb, kb_sb, 1.0 / 1024.0)):
                for c in range(NCH):
                    ps = big_ps("psqkv")
                    for blk in range(D // 256):
                        nc.tensor.matmul(ps[:, :SBLK],
                                         lhsT=w_sb[:, blk, :, c * 128:(c + 1) * 128],
                                         rhs=hT[:, 2 * blk:2 * blk + 2, :],
                                         start=(blk == 0),
                                         stop=(blk == D // 256 - 1), perf_mode=DR)
                    # bias add + bf16 cast + partition shift on ACT
                    nc.scalar.activation(out=dst[:, 2 * c, :], in_=ps[0:64, :SBLK],
                                         func=AF.Identity, bias=b_sb[0:64, c:c + 1],
                                         scale=dsc)
                    nc.scalar.activation(out=dst[:, 2 * c + 1, :],
                                         in_=ps[64:128, :SBLK],
                                         func=AF.Identity,
                                         bias=b_sb[64:128, c:c + 1], scale=dsc)
            # v feature-major (same DoubleRow shape as q/k); per-seq token-major
            # vtok is carved out later by PE transposes
            vT = actpool.tile([128, NCH, SBLK], bf16, tag="vT", name="vT")
            for c in range(NCH):
                ps = big_ps("psv")
                for blk in range(D // 256):
                    nc.tensor.matmul(ps[:, :SBLK],
                                     lhsT=wv_sb[:, blk, :, c * 128:(c + 1) * 128],
                                     rhs=hT[:, 2 * blk:2 * blk + 2, :],
                                     start=(blk == 0),
                                     stop=(blk == D // 256 - 1), perf_mode=DR)
                nc.scalar.activation(out=vT[:, c, :], in_=ps[:, :SBLK],
                                     func=AF.Identity, bias=vb_sb[:, c:c + 1],
                                     scale=1.0 / 1024.0)

            # ---- stage C: attention, software-pipelined across sequences ----
            ctxT = actpool.tile([128, NCH, SBLK], bf16, tag="ctxT", name="ctxT")
            h2T = actpool.tile([128, NCH, SBLK], f8, tag="h2T", name="h2T")
            x2_tiles = []
            e_mvs = []
            a_next = [None, None]   # x_tiles, mvs of superblock isb+1
            next_chunk = 0

            vtoks = [None] * G
            p_sbs = [None] * G
            attnTs = [None] * G

            def emit_v_scores(s):
                so = s * S
                vtok = attnpool.tile([S, H, HD], bf16, tag="vtok", name="vtok",
                                     bufs=3)
                psb = bf_ps("psvtok")
                for c in range(NCH):
                    nc.tensor.transpose(psb[:S, c * 128:(c + 1) * 128],
                                        vT[:, c, so:so + S], ident[:, :])
                nc.any.tensor_copy(out=vtok[:, :, :],
                                   in_=psb[:S, :D].rearrange(
                                       "p (h d) -> p h d", d=HD))
                vtoks[s] = vtok
                p_sb = attnpool.tile([S, H, S], bf16, tag="p", name="p_sb", bufs=2)
                for half in range(2):
                    hh = half * 6
                    ps = pspool.tile([128, 512], f32, tag="scps", name="pssc",
                                     bufs=2)[:S, :468].rearrange(
                                         "p (i k) -> p i k", k=78)
                    for i in range(6):
                        h = hh + i
                        nc.tensor.matmul(ps[:, i, :S],
                                         lhsT=qT[:, h, so:so + S],
                                         rhs=kT[:, h, so:so + S],
                                         start=True, stop=True)
                    # one batched exp for 6 heads
                    nc.scalar.activation(out=p_sb[:, hh:hh + 6, :],
                                         in_=ps[:, :, :S], func=AF.Exp)
                p_sbs[s] = p_sb

            def emit_softmax_chain(s):
                p_sb = p_sbs[s]
                denom = statpool.tile([S, H], f32, tag="denom", name="denom")
                nc.vector.tensor_tensor(
                    out=p_sb[:], in0=p_sb[:],
                    in1=mask_sb[:, None, :].to_broadcast((S, H, S)), op=OP.mult)
                nc.vector.reduce_sum(out=denom[:], in_=p_sb[:], axis=AX.X)
                nc.vector.reciprocal(out=denom[:], in_=denom[:])
                nc.vector.tensor_tensor(
                    out=p_sb[:], in0=p_sb[:],
                    in1=denom[:, :, None].to_broadcast((S, H, S)), op=OP.mult)

            def emit_transposes(s):
                p_sb = p_sbs[s]
                attnT = attnpool.tile([S, H, S], bf16, tag="attnT", name="attnT")
                for half in range(2):
                    hh = half * 6
                    psa = bf_ps("psattnT")[:S, :468].rearrange(
                        "p (i k) -> p i k", k=78)
                    for i in range(6):
                        nc.tensor.transpose(psa[:, i, :S], p_sb[:, hh + i, :],
                                            ident[:S, :S])
                    nc.any.tensor_copy(out=attnT[:, hh:hh + 6, :],
                                       in_=psa[:, :, :S])
                attnTs[s] = attnT

            def emit_ctx(s):
                so = s * S
                vtok, attnT = vtoks[s], attnTs[s]
                # even heads -> ctxT partitions 0:64, odd heads -> 64:128;
                # matmul outputs stay at partition base 0, the copies shift.
                for j in range(2):
                    psc = pspool.tile([128, 512], f32, tag="scps",
                                      name="psctx", bufs=2)[:64, :468].rearrange(
                                          "p (c k) -> p c k", k=78)
                    for c in range(NCH):
                        h = 2 * c + j
                        nc.tensor.matmul(psc[:, c, :S],
                                         lhsT=vtok[:, h, :], rhs=attnT[:, h, :],
                                         start=True, stop=True)
                    nc.vector.tensor_copy(out=ctxT[j * 64:(j + 1) * 64, :,
                                                   so:so + S],
                                          in_=psc[:, :, :S])

            def emit_done_chunks(s_done):
                """emit O-proj + residual + LN2 for chunks fully covered by
                sequences 0..s_done."""
                nonlocal next_chunk
                done_tokens = (s_done + 1) * S
                while (next_chunk < len(chunks)
                       and chunks[next_chunk][0] + chunks[next_chunk][1]
                       <= done_tokens):
                    ci = next_chunk
                    stage_D_chunk(ci, ctxT, x_tiles, x2_tiles)
                    # LN2 stats inline (DVE), so only the short Newton+normalize
                    # chains remain at the superblock seam
                    e_mvs.append(ln_start(x2_tiles[ci], chunks[ci][1], "E"))
                    if isb + 1 < NSB and a_next[0] is None:
                        a_next[0], a_next[1] = stage_A_start(isb + 1)
                    next_chunk += 1

            for s in range(G):
                emit_v_scores(s)
                if s >= 1:
                    emit_transposes(s - 1)
                if s >= 2:
                    emit_ctx(s - 2)
                    emit_done_chunks(s - 2)
                emit_softmax_chain(s)
            emit_transposes(G - 1)
            emit_ctx(G - 2)
            emit_done_chunks(G - 2)
            emit_ctx(G - 1)
            emit_done_chunks(G - 1)

            # superblock seam: finish E (and A') LayerNorms with the finish
            # chains emitted one chunk ahead of the PE transposes they feed
            h2toks = []
            fin = []
            for ci, (coff, w) in enumerate(chunks):
                fin.append(("E", ci))
            if isb + 1 < NSB:
                for ci, (coff, w) in enumerate(chunks):
                    fin.append(("A", ci))
            hT_next = (actpool.tile([128, NCH, SBLK], f8, tag="hT", name="hT",
                                    bufs=2) if isb + 1 < NSB else None)
            a_htoks = []
            done = []
            for j, (kind, ci) in enumerate(fin):
                coff, w = chunks[ci]
                if kind == "E":
                    h2toks.append(ln_finish(e_mvs[ci], x2_tiles[ci], w, "E",
                                            bufs=3, scale16=True))
                else:
                    a_htoks.append(ln_finish(a_next[1][ci], a_next[0][ci], w,
                                             "A", bufs=3, scale16=True))
                if j >= 1:
                    done.append(fin[j - 1])
                    kind2, ci2 = fin[j - 1]
                    coff2, w2 = chunks[ci2]
                    if kind2 == "E":
                        ln_transpose(h2toks[ci2], coff2, w2, h2T, "E")
                    else:
                        ln_transpose(a_htoks[ci2], coff2, w2, hT_next, "A")
            kind2, ci2 = fin[-1]
            coff2, w2 = chunks[ci2]
            if kind2 == "E":
                ln_transpose(h2toks[ci2], coff2, w2, h2T, "E")
            else:
                ln_transpose(a_htoks[ci2], coff2, w2, hT_next, "A")
            if isb + 1 < NSB:
                cur = (hT_next, a_next[0])

            # ---- stage F: MLP ----
            ff1 = actpool.tile([128, NFF, SBLK], f8, tag="ff1", name="ff1")
            DR = mybir.MatmulPerfMode.DoubleRow
            for f in range(NFF):
                ps = big_ps("psff")
                for blk in range(D // 256):
                    nc.tensor.matmul(ps[:, :SBLK],
                                     lhsT=wf1_sb[:, blk, :, f * 128:(f + 1) * 128],
                                     rhs=h2T[:, 2 * blk:2 * blk + 2, :],
                                     start=(blk == 0), stop=(blk == D // 256 - 1),
                                     perf_mode=DR)
                if use_silu:
                    # f1 = silu(1.702*ps + 1.702*b) = 1.702*quickgelu(ps+b);
                    # the 1/1.702 is folded into fc2T host-side.
                    nc.scalar.activation(out=ff1[:, f, :], in_=ps[:, :SBLK],
                                         func=AF.Silu,
                                         bias=f1b_sb[:, f:f + 1],
                                         scale=1.702 / 1024.0)
                else:
                    # CoreSim fallback: sigmoid + 2 DVE ops, same contract
                    sgt = statpool.tile([128, SBLK], bf16, tag="sgt", name="sgt",
                                        bufs=1)
                    nc.scalar.activation(out=sgt, in_=ps[:, :SBLK], func=AF.Sigmoid,
                                         bias=f1b_sb[:, f:f + 1],
                                         scale=1.702 / 1024.0)
                    nc.vector.tensor_scalar(out=ff1[:, f, :], in0=ps[:, :SBLK],
                                            scalar1=1.702 / 1024.0,
                                            scalar2=f1b_sb[:, f:f + 1],
                                            op0=OP.mult, op1=OP.add)
                    nc.vector.tensor_tensor(out=ff1[:, f, :], in0=ff1[:, f, :],
                                            in1=sgt, op=OP.mult)
            # fc2 feature-major: out_fm[c] = sum_f wf2[f,c].T @ ff1[f]
            of2 = actpool.tile([128, NCH, SBLK], bf16, tag="of2", name="of2")
            for c in range(NCH):
                ps = big_ps("psf2")
                for blk in range(FF // 256):
                    nc.tensor.matmul(ps[:, :SBLK],
                                     lhsT=wf2_sb[:, blk, :, c * 128:(c + 1) * 128],
                                     rhs=ff1[:, 2 * blk:2 * blk + 2, :],
                                     start=(blk == 0),
                                     stop=(blk == FF // 256 - 1), perf_mode=DR)
                nc.scalar.activation(out=of2[:, c, :], in_=ps[:, :SBLK],
                                     func=AF.Identity, bias=f2b_sb[:, c:c + 1],
                                     scale=1.0 / 128.0)
            # transpose back to token-major, add residual, store
            for ci, (coff, w) in enumerate(chunks):
                o_tok = outpool.tile([128, D], f32, tag="otok", name="otok")
                for half in range(2):
                    pstr = bf_ps("psf2tr")
                    for j in range(3):
                        c = half * 3 + j
                        nc.tensor.transpose(pstr[:w, j * 128:(j + 1) * 128],
                                            of2[:, c, coff:coff + w],
                                            ident[:, :])
                    sl = slice(half * 384, (half + 1) * 384)
                    nc.vector.tensor_tensor(out=o_tok[:w, sl],
                                            in0=pstr[:w, :384],
                                            in1=x2_tiles[ci][:w, sl], op=OP.add)
                nc.sync.dma_start(out=out_d[t0 + coff: t0 + coff + w, :],
                                  in_=o_tok[:w])

    nc.compile()
    return nc


def prep_shared(inputs):
    """Fold LN affine params / scale constants into weights -> shared in_map entries."""
    bf = ml_dtypes.bfloat16
    f32 = np.float32
    g = {k: np.asarray(v, dtype=np.float32) for k, v in inputs.items() if k != "x"}

    f8w = ml_dtypes.float8_e4m3

    def pack_dr(wT):
        w8 = wT.astype(f8w)
        kdim = w8.shape[0]
        return np.ascontiguousarray(
            w8.reshape(kdim // 256, 2, 128, w8.shape[1]).transpose(2, 0, 1, 3))

    wqT = pack_dr(g["ln1_w"][:, None] * g["qw"].T * 64.0)
    wkT = pack_dr(g["ln1_w"][:, None] * g["kw"].T * 64.0)
    wvT = pack_dr(g["ln1_w"][:, None] * g["vw"].T * 64.0)
    woT = np.ascontiguousarray(g["ow"].T).astype(bf)
    # fc1/fc2 fp8 DoubleRow packing: k = blk*256 + i*128 + p -> [p, blk, i, :]
    fc1T = pack_dr(g["ln2_w"][:, None] * g["fc1_w"].T * 64.0)
    fc2T = pack_dr(g["fc2_w"].T / 1.702 * 128.0)

    qb = ((g["ln1_b"] @ g["qw"].T + g["qb"]) * 0.125).astype(f32)
    kb = (g["ln1_b"] @ g["kw"].T + g["kb"]).astype(f32)
    vb = (g["ln1_b"] @ g["vw"].T + g["vb"]).astype(f32)
    ob = g["ob"].astype(f32)
    fc1b = ((g["ln2_b"] @ g["fc1_w"].T + g["fc1_b"]) * 1.702).astype(f32)
    fc2b = g["fc2_b"].astype(f32)

    mask = np.tril(np.ones((S, S), np.float32)).astype(bf)   # [q, k], k<=q allowed

    return dict(wqT=wqT, wkT=wkT, wvT=wvT, woT=woT, fc1T=fc1T, fc2T=fc2T,
                qb=qb, kb=kb, vb=vb, ob=ob, fc1b=fc1b, fc2b=fc2b, mask=mask)


def prep_host_inputs(inputs):
    shared = prep_shared(inputs)
    x = np.asarray(inputs["x"], dtype=np.float32)
    in_maps = []
    for c in range(N_CORES):
        xc = np.ascontiguousarray(
            x[c * BPC:(c + 1) * BPC].reshape(T_CORE, D).astype(np.float32))
        in_maps.append(dict(shared, x=xc))
    return in_maps


_CACHED_NC = None


def _get_nc():
    global _CACHED_NC
    if _CACHED_NC is None:
        _CACHED_NC = build_program()
    return _CACHED_NC


def run(inputs, trace=False):
    from concourse.bass_utils import run_bass_kernel_spmd
    nc = _get_nc()
    in_maps = prep_host_inputs(inputs)
    res = run_bass_kernel_spmd(nc, in_maps, list(range(N_CORES)), trace=trace)
    outs = [np.asarray(res.results[c]["out"], dtype=np.float32).reshape(BPC, S, D)
            for c in range(N_CORES)]
    full = np.concatenate(outs, axis=0)
    return full, res


def kernel(**inputs):
    full, _ = run(inputs, trace=False)
    return full
